# revision 16
# baseline (speedup 1.0000x reference)
"""Trainium2 Bass kernel for nn_Attention_49907519980190 (v2).

Reference computation (b=2, n=2048, dim=1024, h=16, d=64):
    q = (x @ w_q)   -> (b, h, n, d)
    k, v = split(x @ w_vk)
    dots = (q @ k^T) * sqrt(d)          # NOTE: multiplies by 8
    attn = softmax(dots)
    out = (attn @ v) reassembled -> (b, n, h*d) @ w_out

Sharding (8 cores): batch x head-group parallel. Core c handles batch
b = c // 4 and heads 4*(c % 4) .. 4*(c % 4) + 4. Column-parallel
q/k/v projections, row-parallel out projection; the host sums the four
partial outputs per batch.

v2 design (vs baseline):
- Single-pass fp16 projections (emulated rel err 0.0045 vs 2e-2 gate;
  the bf16 hi/lo 3-pass of the baseline was overkill).
- x^T produced by fp16 cast + DMA transpose (sync queue) instead of
  128 PE transposes + ACT copies.
- Q^T/K^T stored pair-stacked: partition = d-of-head-pair (head 2m in
  rows 0:64, head 2m+1 in 64:128): full-width PSUM evacuations and
  K=64 dots matmuls (auto tile_position from base partition).
- Softmax max via ONE DVE tensor_tensor_reduce per (h, it):
  bias8 = min((Sh0 max Sh1) * -8) = -8 * rowmax. Full max (subset max
  is numerically catastrophic here: logit sigma ~64).
- exp on ACT in two [128, 1024] instructions per (h, it) with
  accum_out giving the softmax denominator for free.
- PV reoriented: V tile as stationary ([128 k, 64]), P^T as moving
  (N=512), producing O^T[d, q] directly in PSUM (col-tiled per head
  pair: head 2m -> out partitions 0:64, 2m+1 -> 64:128). Kills the
  baseline's 1024 LDW-bound N=65 matmuls and the phase-E PE
  transposes. Denominator reciprocals are transposed to a row via a
  small DMA gather, partition-broadcast on gpsimd, and applied in one
  DVE multiply per pair during PSUM evacuation.
- PSUM: dots S in [128, 1024] halves (2 banks, bufs=2) + O^T pair
  accumulator [128, 2048] (4 banks) = 8 banks exactly.
"""

import numpy as np

import concourse.bass as bass
import concourse.mybir as mybir
import concourse.tile as tile
from concourse import bacc
from concourse.bass_utils import run_bass_kernel_spmd

F32 = mybir.dt.float32
BF16 = mybir.dt.bfloat16
FP16 = mybir.dt.float16
ADD = mybir.AluOpType.add
MULT = mybir.AluOpType.mult
MAX = mybir.AluOpType.max
MIN = mybir.AluOpType.min
AX = mybir.AxisListType.X
EXP = mybir.ActivationFunctionType.Exp

P = 128      # partitions
NTOK = 2048  # tokens per core (one batch slice)
DIM = 1024   # model dim
E = 256      # per-core projection width (4 heads x 64)
NH = 4       # heads per core
D = 64       # head dim
KO = 8       # contraction chunks of 128 over DIM
TT = 16      # token tiles of 128
SCALE = 8.0  # sqrt(D); reference MULTIPLIES by it
FLT_BIG = 3.0e38


def build_attention_nc():
    nc = bacc.Bacc("TRN2", target_bir_lowering=False, debug=False)

    x = nc.declare_dram_parameter("x", [NTOK, DIM], F32, isOutput=False)
    wq = nc.declare_dram_parameter("wq", [DIM, E], F32, isOutput=False)
    wk = nc.declare_dram_parameter("wk", [DIM, E], F32, isOutput=False)
    wv = nc.declare_dram_parameter("wv", [DIM, E], F32, isOutput=False)
    wo = nc.declare_dram_parameter("wo", [E, DIM], F32, isOutput=False)
    y = nc.declare_dram_parameter("y", [NTOK, DIM], F32, isOutput=True)

    with tile.TileContext(nc) as tc:
        with tc.tile_pool(name="persist", bufs=1) as persist:
            # Q^T / K^T pair-stacked: partition = d of head pair
            # (head 2m rows 0:64, head 2m+1 rows 64:128), free = (pair, tok)
            QT2 = persist.tile([P, 2, NTOK], FP16)
            KT2 = persist.tile([P, 2, NTOK], FP16)
            # V natural: [tok_low, tok_tile, e]
            Vb = persist.tile([P, TT, E], FP16)
            wo16 = persist.tile([P, 2, DIM], FP16)
            # O^T pair-stacked fp16: partition = e of pair, free = (pair, tok)
            OT = persist.tile([P, 2, NTOK], FP16)
            # softmax denominators per (head, it), q on partitions
            den4 = persist.tile([P, NH, TT], F32)
            # reciprocal rows: partition hh holds head hh's 1/denom over q
            recT = persist.tile([P, NTOK], FP16)

            # ---------- Phase A: weights + x^T (cast + DMA transpose)
            with tc.tile_pool(name="xw", bufs=1) as xw:
                xT = xw.tile([P, KO, NTOK], FP16)  # x^T: [d_low, d_chunk, tok]
                wq16 = xw.tile([P, KO, E], FP16)
                wk16 = xw.tile([P, KO, E], FP16)
                wv16 = xw.tile([P, KO, E], FP16)

                with tc.tile_pool(name="stage", bufs=1) as stage:
                    for wsrc, wdst in ((wk, wk16), (wq, wq16), (wv, wv16)):
                        wf = stage.tile([P, KO, E], F32, tag="wf", bufs=2)
                        nc.scalar.dma_start(
                            out=wf,
                            in_=wsrc[:, :].rearrange("(ko p) e -> p ko e", p=P),
                        )
                        nc.scalar.copy(out=wdst, in_=wf)
                    wof = stage.tile([P, 2, DIM], F32, tag="wof", bufs=1)
                    nc.scalar.dma_start(
                        out=wof, in_=wo[:, :].rearrange("(eo p) d -> p eo d", p=P)
                    )
                    nc.scalar.copy(out=wo16, in_=wof)

                    for tt in range(TT):
                        ts = slice(tt * P, (tt + 1) * P)
                        xf = stage.tile([P, DIM], F32, tag="xf", bufs=3)
                        ldq = nc.gpsimd if tt % 2 == 0 else nc.scalar
                        ldq.dma_start(out=xf, in_=x[ts, :])
                        xc = stage.tile([P, DIM], FP16, tag="xc", bufs=3)
                        ceng = nc.vector if tt % 2 == 0 else nc.gpsimd
                        ceng.tensor_copy(out=xc, in_=xf)
                        nc.sync.dma_start_transpose(out=xT[:, :, ts], in_=xc)

                    # ---------- Phase C: projections (single-pass fp16)
                    with tc.tile_pool(name="psA", bufs=1, space="PSUM") as psA:
                        # K then Q for pair 0 first, V in between, pair 1 last
                        def proj(w16, dst, m):
                            ms = slice(m * P, (m + 1) * P)
                            pr4 = psA.tile([P, 4, 512], F32, tag="pr4", bufs=1)
                            for g in range(4):
                                for c in range(KO):
                                    nc.tensor.matmul(
                                        pr4[:, g, :], w16[:, c, ms],
                                        xT[:, c, g * 512:(g + 1) * 512],
                                        start=(c == 0), stop=(c == KO - 1),
                                    )
                            nc.scalar.copy(
                                out=dst[:, m, :],
                                in_=pr4.rearrange("p g n -> p (g n)"),
                            )

                        proj(wk16, KT2, 0)
                        proj(wq16, QT2, 0)
                        for tm in range(TT):
                            tms = slice(tm * P, (tm + 1) * P)
                            prv = psA.tile([P, E], F32, tag="prv", bufs=2)
                            for c in range(KO):
                                nc.tensor.matmul(
                                    prv, xT[:, c, tms], wv16[:, c, :],
                                    start=(c == 0), stop=(c == KO - 1),
                                )
                            nc.vector.tensor_copy(out=Vb[:, tm, :], in_=prv)
                        proj(wk16, KT2, 1)
                        proj(wq16, QT2, 1)

            # ---------- Phase D: attention
            with (
                tc.tile_pool(name="psS", bufs=2, space="PSUM") as psS,
                tc.tile_pool(name="psO", bufs=1, space="PSUM") as psO,
                tc.tile_pool(name="attn_sb", bufs=1) as attn_sb,
                tc.tile_pool(name="attn_small", bufs=1) as attn_small,
            ):
                # PT4[itg]: P^T for 4 q-tiles: [k_low, k_tile, q(512)]
                pt4s = [[None] * 4 for _ in range(NH)]
                o_ps = [None, None]   # per-pair O^T PSUM accumulator
                recb = [None, None]   # per-pair reciprocal broadcast

                def issue_pv_group(h, itg):
                    row = (h % 2) * D
                    hs = slice(h * D, (h + 1) * D)
                    for jo in range(TT):
                        nc.tensor.matmul(
                            o_ps[h // 2][row:row + D, itg * 512:(itg + 1) * 512],
                            Vb[:, jo, hs],
                            pt4s[h][itg][:, jo, :],
                            start=(jo == 0), stop=(jo == TT - 1),
                        )

                def prep_rec_head(hh):
                    # replicate recT row to 64 partitions by doubling DMAs
                    rb = recb[hh // 2]
                    base = (hh % 2) * D
                    dq = nc.gpsimd if hh % 2 == 0 else nc.scalar
                    dq.dma_start(
                        out=rb[base:base + 1, :], in_=recT[32 * hh:32 * hh + 1, :]
                    )
                    w = 1
                    while w < D:
                        dq.dma_start(
                            out=rb[base + w:base + 2 * w, :],
                            in_=rb[base:base + w, :],
                        )
                        w *= 2

                def evac_pair(pair):
                    nc.vector.tensor_tensor(
                        out=OT[:, pair, :], in0=o_ps[pair], in1=recb[pair],
                        op=MULT,
                    )

                for h in range(NH):
                    m = h // 2
                    row = (h % 2) * D
                    if h % 2 == 0:
                        o_ps[m] = psO.tile([P, NTOK], F32, tag="O", bufs=1, name="o_ps")
                        recb[m] = attn_sb.tile(
                            [P, NTOK], FP16, tag="recb", bufs=2, name="recb_t"
                        )
                    for it in range(TT):
                        isl = slice(it * P, (it + 1) * P)
                        halves = []
                        for half in range(2):
                            sh = psS.tile([P, 1024], F32, tag="Sh", bufs=2)
                            for nn in range(2):
                                ks = slice(half * 1024 + nn * 512,
                                           half * 1024 + (nn + 1) * 512)
                                nc.tensor.matmul(
                                    sh[:, nn * 512:(nn + 1) * 512],
                                    QT2[row:row + D, m, isl],
                                    KT2[row:row + D, m, ks],
                                    start=True, stop=True,
                                )
                            halves.append(sh)
                        # bias8 = -8 * rowmax via negated per-half maxes
                        # (DVE may read only one PSUM input per instruction)
                        nm01 = attn_small.tile([P, 2], F32, tag="nm01", bufs=4)
                        for half in range(2):
                            nc.vector.tensor_reduce(
                                out=nm01[:, half:half + 1], in_=halves[half],
                                axis=AX, op=MAX, negate=True,
                            )
                        nmn = attn_small.tile([P, 1], F32, tag="nmn", bufs=4)
                        nc.vector.tensor_reduce(
                            out=nmn, in_=nm01, axis=AX, op=MIN,
                        )
                        bias8 = attn_small.tile([P, 1], F32, tag="bias8", bufs=4)
                        nc.gpsimd.tensor_scalar_mul(bias8, nmn, SCALE)
                        u_t = attn_sb.tile([P, NTOK], FP16, tag="u", bufs=4)
                        denh = attn_small.tile([P, 2], F32, tag="denh", bufs=4)
                        for half in range(2):
                            nc.scalar.activation(
                                out=u_t[:, half * 1024:(half + 1) * 1024],
                                in_=halves[half], func=EXP,
                                bias=bias8, scale=SCALE,
                                accum_out=denh[:, half:half + 1],
                            )
                        nc.vector.tensor_reduce(
                            out=den4[:, h, it:it + 1], in_=denh,
                            axis=AX, op=ADD,
                        )
                        # reciprocal column, then a tiny DMA "transpose"
                        # (partition-major read -> one row of recT)
                        reccol = attn_small.tile([P, 1], FP16, tag="rc", bufs=4)
                        with nc.allow_low_precision(reason="fp16 softmax recip"):
                            nc.vector.reciprocal(
                                out=reccol, in_=den4[:, h, it:it + 1]
                            )
                        rdq = nc.scalar if it % 2 == 0 else nc.gpsimd
                        rdq.dma_start(
                            out=recT[32 * h:32 * h + 1, isl], in_=reccol
                        )
                        itg, itq = it // 4, it % 4
                        if itq == 0:
                            pt4s[h][itg] = attn_sb.tile(
                                [P, TT, 512], FP16, tag="PT4", bufs=6,
                                name="pt4",
                            )
                        nc.sync.dma_start_transpose(
                            out=pt4s[h][itg][:, :, itq * P:(itq + 1) * P],
                            in_=u_t,
                        )
                        # PV for the previous 4-tile q-group (one it of lag so
                        # the PE never stalls on the just-issued transpose)
                        if itq == 0 and itg > 0:
                            issue_pv_group(h, itg - 1)
                    issue_pv_group(h, 3)
                    prep_rec_head(h)
                    if h % 2 == 1:
                        evac_pair(h // 2)

            # ---------- Phase E: y = O @ wo from O^T directly
            with (
                tc.tile_pool(name="psE", bufs=2, space="PSUM") as psE,
                tc.tile_pool(name="ysb", bufs=1) as ysb,
            ):
                for tm in range(TT):
                    ms = slice(tm * P, (tm + 1) * P)
                    for n in range(2):
                        ns = slice(n * 512, (n + 1) * 512)
                        yp = psE.tile([P, 512], F32, tag="yp", bufs=2)
                        for eo in range(2):
                            nc.tensor.matmul(
                                yp[:, :], OT[:, eo, ms], wo16[:, eo, ns],
                                start=(eo == 0), stop=(eo == 1),
                            )
                        yo = ysb.tile([P, 512], F32, tag="yo", bufs=4)
                        nc.vector.tensor_copy(out=yo, in_=yp)
                        eng = nc.sync if (tm + n) % 2 == 0 else nc.scalar
                        eng.dma_start(out=y[ms, ns], in_=yo)

    nc.compile()
    return nc


_NC_CACHE = None


def _get_nc():
    global _NC_CACHE
    if _NC_CACHE is None:
        _NC_CACHE = build_attention_nc()
    return _NC_CACHE


def kernel(x, w_q, w_vk, w_out, **run_kwargs):
    """Full inputs in, full output out. Shards over 8 NeuronCores."""
    b, n, dim = x.shape
    assert (b, n, dim) == (2, 2048, 1024)
    w_k = w_vk[:, :1024]
    w_v = w_vk[:, 1024:]

    in_maps = []
    for c in range(8):
        bi = c // 4
        hg = c % 4
        cs = slice(hg * E, (hg + 1) * E)
        in_maps.append({
            "x": np.ascontiguousarray(x[bi]).astype(np.float32),
            "wq": np.ascontiguousarray(w_q[:, cs]).astype(np.float32),
            "wk": np.ascontiguousarray(w_k[:, cs]).astype(np.float32),
            "wv": np.ascontiguousarray(w_v[:, cs]).astype(np.float32),
            "wo": np.ascontiguousarray(w_out[cs, :]).astype(np.float32),
        })

    nc = _get_nc()
    res = run_bass_kernel_spmd(nc, in_maps, core_ids=list(range(8)), **run_kwargs)
    out = np.zeros((2, 2048, 1024), dtype=np.float32)
    for c in range(8):
        out[c // 4] += res.results[c]["y"]
    if run_kwargs:
        kernel.last_results = res
    return out


# revision 21
# speedup vs baseline: 1.0480x; 1.0480x over previous
"""Trainium2 Bass kernel for nn_Attention_49907519980190 (v2).

Reference computation (b=2, n=2048, dim=1024, h=16, d=64):
    q = (x @ w_q)   -> (b, h, n, d)
    k, v = split(x @ w_vk)
    dots = (q @ k^T) * sqrt(d)          # NOTE: multiplies by 8
    attn = softmax(dots)
    out = (attn @ v) reassembled -> (b, n, h*d) @ w_out

Sharding (8 cores): batch x head-group parallel. Core c handles batch
b = c // 4 and heads 4*(c % 4) .. 4*(c % 4) + 4. Column-parallel
q/k/v projections, row-parallel out projection; the host sums the four
partial outputs per batch.

v2 design (vs baseline):
- Single-pass fp16 projections (emulated rel err 0.0045 vs 2e-2 gate;
  the bf16 hi/lo 3-pass of the baseline was overkill).
- x^T produced by fp16 cast + DMA transpose (sync queue) instead of
  128 PE transposes + ACT copies.
- Q^T/K^T stored pair-stacked: partition = d-of-head-pair (head 2m in
  rows 0:64, head 2m+1 in 64:128): full-width PSUM evacuations and
  K=64 dots matmuls (auto tile_position from base partition).
- Softmax max via ONE DVE tensor_tensor_reduce per (h, it):
  bias8 = min((Sh0 max Sh1) * -8) = -8 * rowmax. Full max (subset max
  is numerically catastrophic here: logit sigma ~64).
- exp on ACT in two [128, 1024] instructions per (h, it) with
  accum_out giving the softmax denominator for free.
- PV reoriented: V tile as stationary ([128 k, 64]), P^T as moving
  (N=512), producing O^T[d, q] directly in PSUM (col-tiled per head
  pair: head 2m -> out partitions 0:64, 2m+1 -> 64:128). Kills the
  baseline's 1024 LDW-bound N=65 matmuls and the phase-E PE
  transposes. Denominator reciprocals are transposed to a row via a
  small DMA gather, partition-broadcast on gpsimd, and applied in one
  DVE multiply per pair during PSUM evacuation.
- PSUM: dots S in [128, 1024] halves (2 banks, bufs=2) + O^T pair
  accumulator [128, 2048] (4 banks) = 8 banks exactly.
"""

import numpy as np

import concourse.bass as bass
import concourse.mybir as mybir
import concourse.tile as tile
from concourse import bacc
from concourse.bass_utils import run_bass_kernel_spmd

F32 = mybir.dt.float32
BF16 = mybir.dt.bfloat16
FP16 = mybir.dt.float16
ADD = mybir.AluOpType.add
MULT = mybir.AluOpType.mult
MAX = mybir.AluOpType.max
MIN = mybir.AluOpType.min
AX = mybir.AxisListType.X
EXP = mybir.ActivationFunctionType.Exp

P = 128      # partitions
NTOK = 2048  # tokens per core (one batch slice)
DIM = 1024   # model dim
E = 256      # per-core projection width (4 heads x 64)
NH = 4       # heads per core
D = 64       # head dim
KO = 8       # contraction chunks of 128 over DIM
TT = 16      # token tiles of 128
SCALE = 8.0  # sqrt(D); reference MULTIPLIES by it
FLT_BIG = 3.0e38


def build_attention_nc():
    nc = bacc.Bacc("TRN2", target_bir_lowering=False, debug=False)

    x = nc.declare_dram_parameter("x", [NTOK, DIM], F32, isOutput=False)
    wq = nc.declare_dram_parameter("wq", [DIM, E], F32, isOutput=False)
    wk = nc.declare_dram_parameter("wk", [DIM, E], F32, isOutput=False)
    wv = nc.declare_dram_parameter("wv", [DIM, E], F32, isOutput=False)
    wo = nc.declare_dram_parameter("wo", [E, DIM], F32, isOutput=False)
    y = nc.declare_dram_parameter("y", [NTOK, DIM], F32, isOutput=True)

    with tile.TileContext(nc) as tc:
        with tc.tile_pool(name="persist", bufs=1) as persist:
            # Q^T / K^T pair-stacked: partition = d of head pair
            # (head 2m rows 0:64, head 2m+1 rows 64:128), free = (pair, tok)
            QT2 = persist.tile([P, 2, NTOK], FP16)
            KT2 = persist.tile([P, 2, NTOK], FP16)
            # V natural: [tok_low, tok_tile, e]
            Vb = persist.tile([P, TT, E], FP16)
            wo16 = persist.tile([P, 2, DIM], FP16)
            # O^T pair-stacked fp16: partition = e of pair, free = (pair, tok)
            OT = persist.tile([P, 2, NTOK], FP16)
            # unnormalized O^T, evacuated per 512-token group
            OTraw = persist.tile([P, 2, NTOK], FP16)
            # softmax denominators per (head, it), q on partitions
            den4 = persist.tile([P, NH, TT], F32)
            # reciprocal rows: partition hh holds head hh's 1/denom over q
            recT = persist.tile([P, NTOK], FP16)

            # ---------- Phase A: weights + x^T (cast + DMA transpose)
            with tc.tile_pool(name="xw", bufs=1) as xw:
                xT = xw.tile([P, KO, NTOK], FP16)  # x^T: [d_low, d_chunk, tok]
                wq16 = xw.tile([P, KO, E], FP16)
                wk16 = xw.tile([P, KO, E], FP16)
                wv16 = xw.tile([P, KO, E], FP16)

                with tc.tile_pool(name="stage", bufs=1) as stage:
                    for wsrc, wdst in ((wk, wk16), (wq, wq16), (wv, wv16)):
                        wf = stage.tile([P, KO, E], F32, tag="wf", bufs=2)
                        nc.scalar.dma_start(
                            out=wf,
                            in_=wsrc[:, :].rearrange("(ko p) e -> p ko e", p=P),
                        )
                        nc.scalar.copy(out=wdst, in_=wf)
                    wof = stage.tile([P, 2, DIM], F32, tag="wof", bufs=1)
                    nc.scalar.dma_start(
                        out=wof, in_=wo[:, :].rearrange("(eo p) d -> p eo d", p=P)
                    )
                    nc.scalar.copy(out=wo16, in_=wof)

                    for tt in range(TT):
                        ts = slice(tt * P, (tt + 1) * P)
                        xf = stage.tile([P, DIM], F32, tag="xf", bufs=3)
                        ldq = nc.gpsimd if tt % 2 == 0 else nc.scalar
                        ldq.dma_start(out=xf, in_=x[ts, :])
                        xc = stage.tile([P, DIM], FP16, tag="xc", bufs=3)
                        ceng = nc.vector if tt % 2 == 0 else nc.gpsimd
                        ceng.tensor_copy(out=xc, in_=xf)
                        nc.sync.dma_start_transpose(out=xT[:, :, ts], in_=xc)

                    # ---------- Phase C: projections (single-pass fp16)
                    with tc.tile_pool(name="psA", bufs=1, space="PSUM") as psA:
                        # K then Q for pair 0 first, V in between, pair 1 last
                        def proj(w16, dst, m, scale=1.0):
                            ms = slice(m * P, (m + 1) * P)
                            pr4 = psA.tile([P, 4, 512], F32, tag="pr4", bufs=1)
                            for g in range(4):
                                for c in range(KO):
                                    nc.tensor.matmul(
                                        pr4[:, g, :], w16[:, c, ms],
                                        xT[:, c, g * 512:(g + 1) * 512],
                                        start=(c == 0), stop=(c == KO - 1),
                                    )
                            # Q is pre-scaled by 8 so the dots come out as the
                            # final logits and the exp bias is just -rowmax
                            nc.scalar.mul(
                                out=dst[:, m, :],
                                in_=pr4.rearrange("p g n -> p (g n)"),
                                mul=scale,
                            )

                        proj(wk16, KT2, 0)
                        proj(wq16, QT2, 0, scale=SCALE)
                        for tm in range(TT):
                            tms = slice(tm * P, (tm + 1) * P)
                            prv = psA.tile([P, E], F32, tag="prv", bufs=2)
                            for c in range(KO):
                                nc.tensor.matmul(
                                    prv, xT[:, c, tms], wv16[:, c, :],
                                    start=(c == 0), stop=(c == KO - 1),
                                )
                            nc.vector.tensor_copy(out=Vb[:, tm, :], in_=prv)
                        proj(wk16, KT2, 1)
                        proj(wq16, QT2, 1, scale=SCALE)

            # ---------- Phase D: attention
            with (
                tc.tile_pool(name="psS", bufs=2, space="PSUM") as psS,
                tc.tile_pool(name="psO", bufs=1, space="PSUM") as psO,
                tc.tile_pool(name="attn_sb", bufs=1) as attn_sb,
                tc.tile_pool(name="attn_small", bufs=1) as attn_small,
            ):
                # PT4[itg]: P^T for 4 q-tiles: [k_low, k_tile, q(512)]
                pt4s = [[None] * 4 for _ in range(NH)]
                recb = [None, None]   # per-pair reciprocal broadcast

                def issue_pv_group(h, itg):
                    row = (h % 2) * D
                    hs = slice(h * D, (h + 1) * D)
                    o_g = psO.tile([P, 512], F32, tag="O", bufs=2, name="o_g")
                    for jo in range(TT):
                        nc.tensor.matmul(
                            o_g[row:row + D, :],
                            Vb[:, jo, hs],
                            pt4s[h][itg][:, jo, :],
                            start=(jo == 0), stop=(jo == TT - 1),
                        )
                    nc.vector.tensor_copy(
                        out=OTraw[row:row + D, h // 2,
                                  itg * 512:(itg + 1) * 512],
                        in_=o_g[row:row + D, :],
                    )

                def prep_rec_head(hh):
                    # replicate recT row to 64 partitions by doubling DMAs
                    rb = recb[hh // 2]
                    base = (hh % 2) * D
                    dq = nc.gpsimd if hh % 2 == 0 else nc.scalar
                    dq.dma_start(
                        out=rb[base:base + 1, :], in_=recT[32 * hh:32 * hh + 1, :]
                    )
                    w = 1
                    while w < D:
                        dq.dma_start(
                            out=rb[base + w:base + 2 * w, :],
                            in_=rb[base:base + w, :],
                        )
                        w *= 2

                def evac_pair(pair):
                    nc.vector.tensor_tensor(
                        out=OT[:, pair, :], in0=OTraw[:, pair, :],
                        in1=recb[pair], op=MULT,
                    )

                for h in range(NH):
                    m = h // 2
                    row = (h % 2) * D
                    if h % 2 == 0:
                        recb[m] = attn_sb.tile(
                            [P, NTOK], FP16, tag="recb", bufs=2, name="recb_t"
                        )
                    for it in range(TT):
                        isl = slice(it * P, (it + 1) * P)
                        halves = []
                        for half in range(2):
                            sh = psS.tile([P, 1024], F32, tag="Sh", bufs=3)
                            for nn in range(2):
                                ks = slice(half * 1024 + nn * 512,
                                           half * 1024 + (nn + 1) * 512)
                                nc.tensor.matmul(
                                    sh[:, nn * 512:(nn + 1) * 512],
                                    QT2[row:row + D, m, isl],
                                    KT2[row:row + D, m, ks],
                                    start=True, stop=True,
                                )
                            halves.append(sh)
                        # bias8 = -8 * rowmax via negated per-half maxes
                        # (DVE may read only one PSUM input per instruction)
                        nm01 = attn_small.tile([P, 2], F32, tag="nm01", bufs=6)
                        for half in range(2):
                            # read only the high 16 bits of each fp32 (= bf16
                            # truncation): 2x DVE rate, max off by < 2^-8 rel
                            hi = halves[half].bitcast(BF16).rearrange(
                                "p (n two) -> p two n", two=2
                            )[:, 1:2, :]
                            nc.vector.tensor_reduce(
                                out=nm01[:, half:half + 1], in_=hi,
                                axis=AX, op=MAX, negate=True,
                            )
                        nmn = attn_small.tile([P, 1], F32, tag="nmn", bufs=6)
                        nc.vector.tensor_reduce(
                            out=nmn, in_=nm01, axis=AX, op=MIN,
                        )
                        u_t = attn_sb.tile([P, NTOK], FP16, tag="u", bufs=6)
                        denh = attn_small.tile([P, 2], F32, tag="denh", bufs=4)
                        for half in range(2):
                            nc.scalar.activation(
                                out=u_t[:, half * 1024:(half + 1) * 1024],
                                in_=halves[half], func=EXP,
                                bias=nmn, scale=1.0,
                                accum_out=denh[:, half:half + 1],
                            )
                        nc.vector.tensor_reduce(
                            out=den4[:, h, it:it + 1], in_=denh,
                            axis=AX, op=ADD,
                        )
                        # reciprocal column, then a tiny DMA "transpose"
                        # (partition-major read -> one row of recT)
                        reccol = attn_small.tile([P, 1], FP16, tag="rc", bufs=4)
                        with nc.allow_low_precision(reason="fp16 softmax recip"):
                            nc.vector.reciprocal(
                                out=reccol, in_=den4[:, h, it:it + 1]
                            )
                        rdq = nc.scalar if it % 2 == 0 else nc.gpsimd
                        rdq.dma_start(
                            out=recT[32 * h:32 * h + 1, isl], in_=reccol
                        )
                        itg, itq = it // 4, it % 4
                        if itq == 0:
                            pt4s[h][itg] = attn_sb.tile(
                                [P, TT, 512], FP16, tag="PT4", bufs=6,
                                name="pt4",
                            )
                        nc.sync.dma_start_transpose(
                            out=pt4s[h][itg][:, :, itq * P:(itq + 1) * P],
                            in_=u_t,
                        )
                        # PV for the previous 4-tile q-group (one it of lag so
                        # the PE never stalls on the just-issued transpose)
                        if itq == 0 and itg > 0:
                            issue_pv_group(h, itg - 1)
                    issue_pv_group(h, 3)
                    prep_rec_head(h)
                    if h % 2 == 1:
                        evac_pair(h // 2)

            # ---------- Phase E: y = O @ wo from O^T directly
            with (
                tc.tile_pool(name="psE", bufs=2, space="PSUM") as psE,
                tc.tile_pool(name="ysb", bufs=1) as ysb,
            ):
                for tm in range(TT):
                    ms = slice(tm * P, (tm + 1) * P)
                    for n in range(2):
                        ns = slice(n * 512, (n + 1) * 512)
                        yp = psE.tile([P, 512], F32, tag="yp", bufs=2)
                        for eo in range(2):
                            nc.tensor.matmul(
                                yp[:, :], OT[:, eo, ms], wo16[:, eo, ns],
                                start=(eo == 0), stop=(eo == 1),
                            )
                        yo = ysb.tile([P, 512], F32, tag="yo", bufs=4)
                        nc.vector.tensor_copy(out=yo, in_=yp)
                        eng = nc.sync if (tm + n) % 2 == 0 else nc.scalar
                        eng.dma_start(out=y[ms, ns], in_=yo)

    nc.compile()
    return nc


_NC_CACHE = None


def _get_nc():
    global _NC_CACHE
    if _NC_CACHE is None:
        _NC_CACHE = build_attention_nc()
    return _NC_CACHE


def kernel(x, w_q, w_vk, w_out, **run_kwargs):
    """Full inputs in, full output out. Shards over 8 NeuronCores."""
    b, n, dim = x.shape
    assert (b, n, dim) == (2, 2048, 1024)
    w_k = w_vk[:, :1024]
    w_v = w_vk[:, 1024:]

    in_maps = []
    for c in range(8):
        bi = c // 4
        hg = c % 4
        cs = slice(hg * E, (hg + 1) * E)
        in_maps.append({
            "x": np.ascontiguousarray(x[bi]).astype(np.float32),
            "wq": np.ascontiguousarray(w_q[:, cs]).astype(np.float32),
            "wk": np.ascontiguousarray(w_k[:, cs]).astype(np.float32),
            "wv": np.ascontiguousarray(w_v[:, cs]).astype(np.float32),
            "wo": np.ascontiguousarray(w_out[cs, :]).astype(np.float32),
        })

    nc = _get_nc()
    res = run_bass_kernel_spmd(nc, in_maps, core_ids=list(range(8)), **run_kwargs)
    out = np.zeros((2, 2048, 1024), dtype=np.float32)
    for c in range(8):
        out[c // 4] += res.results[c]["y"]
    if run_kwargs:
        kernel.last_results = res
    return out
